# revision 4
# baseline (speedup 1.0000x reference)
"""Joint attention layer on 8 trn2 NeuronCores (query-sharded, SPMD).

Math (reference):
    Q = img @ Wq.T ; K = text @ Wk.T ; S = Q @ K.T        [N, N]
    attn = softmax(S, axis=1) / sqrt(D)
    out_img = attn @ img ; out_text = attn @ text

Per-core plan (core c owns query rows m in [c*1024, (c+1)*1024)):
    H[j,i]  = sum_d Wq[d,j] Wk[d,i]              (= Wq.T @ Wk, 256x256)
    G[i,m]  = sum_j H[j,i] imgT[j,m]             (absorbs both projections)
    S^T[n,m] = sum_i text[n,i] G[i,m]            (keys on partitions)
    P^T = exp(S^T)  (no max subtraction needed: |S| <~ 35 << 88)
    O[m,:] = sum_n P^T[n,m] * [img|text][n,:]    (PSUM accum over all n)
    rowsum[m] = sum_n P^T[n,m]                   (ones-vector matmul)
    out[m,:] = O[m,:] / rowsum[m] / sqrt(D)

Host passes img/text both natural and transposed so the kernel never
transposes on device. No collectives: outputs are disjoint row slabs.
"""

import numpy as np
from contextlib import ExitStack

import concourse.bass as bass
import concourse.tile as tile
from concourse import bacc, mybir
from concourse.bass_utils import run_bass_kernel_spmd

F32 = mybir.dt.float32
P = 128          # partitions
D = 256          # hidden dim
N = 8192         # sequence length
N_CORES = 8
SLAB = N // N_CORES          # 1024 query rows per core
MB = 2                       # m-blocks per core
MBS = SLAB // MB             # 512 queries per m-block
NSUB = MBS // P              # 4 psum subtiles per m-block
NCH = N // P                 # 64 key chunks of 128
TTG = 8                      # textT column-group tiles
TTW = N // TTG               # 1024 cols per group
PIPE = 2                     # S-stage lookahead (chunks)
NORM = 1.0 / 16.0            # 1/sqrt(D)

_CACHE = {}


def _build_nc(debug=False):
    nc = bacc.Bacc("TRN2", target_bir_lowering=False, debug=False,
                   num_devices=N_CORES)

    img_d = nc.dram_tensor("img", [N, D], F32, kind="ExternalInput").ap()
    text_d = nc.dram_tensor("text", [N, D], F32, kind="ExternalInput").ap()
    textT_d = nc.dram_tensor("textT", [D, N], F32, kind="ExternalInput").ap()
    imgT_d = nc.dram_tensor("imgT_slab", [D, SLAB], F32, kind="ExternalInput").ap()
    wq_d = nc.dram_tensor("Wq", [D, D], F32, kind="ExternalInput").ap()
    wk_d = nc.dram_tensor("Wk", [D, D], F32, kind="ExternalInput").ap()
    out_d = nc.dram_tensor("out", [SLAB, 2 * D], F32, kind="ExternalOutput").ap()
    if debug:
        dbg_g = nc.dram_tensor("dbg_g", [D, SLAB], F32, kind="ExternalOutput").ap()
        dbg_pt = nc.dram_tensor("dbg_pt", [P, MBS], F32, kind="ExternalOutput").ap()
        dbg_rs = nc.dram_tensor("dbg_rs", [P, NSUB], F32, kind="ExternalOutput").ap()

    with tile.TileContext(nc) as tc:
        with ExitStack() as ctx:
            const = ctx.enter_context(tc.tile_pool(name="const", bufs=1))

            wq_sb = [const.tile([P, D], F32, tag=f"wq{t}", name=f"wq{t}") for t in range(2)]
            wk_sb = [const.tile([P, D], F32, tag=f"wk{t}", name=f"wk{t}") for t in range(2)]
            imgT_sb = [const.tile([P, SLAB], F32, tag=f"imgT{t}", name=f"imgT{t}") for t in range(2)]
            for t in range(2):
                nc.sync.dma_start(wq_sb[t][:], wq_d[t * P:(t + 1) * P, :])
                nc.sync.dma_start(wk_sb[t][:], wk_d[t * P:(t + 1) * P, :])
                nc.sync.dma_start(imgT_sb[t][:], imgT_d[t * P:(t + 1) * P, :])

            tt_sb = [[const.tile([P, TTW], F32, tag=f"tt{it}_{g}", name=f"tt{it}_{g}")
                      for g in range(TTG)] for it in range(2)]
            for it in range(2):
                for g in range(TTG):
                    nc.sync.dma_start(
                        tt_sb[it][g][:],
                        textT_d[it * P:(it + 1) * P, g * TTW:(g + 1) * TTW])

            ones_sb = const.tile([P, 1], F32, tag="ones", name="ones")
            nc.vector.memset(ones_sb[:], 1.0)

            h_sb = [const.tile([P, D], F32, tag=f"h{jt}", name=f"h{jt}") for jt in range(2)]
            g_sb = [const.tile([P, SLAB], F32, tag=f"g{it}", name=f"g{it}") for it in range(2)]

            # ---- setup: H = Wq.T @ Wk ; G = H.T-contract @ imgT ----
            with tc.tile_pool(name="psetup", bufs=2, space="PSUM") as psetup:
                for jt in range(2):
                    hp = psetup.tile([P, D], F32, tag="h", name=f"hp{jt}")
                    for dt in range(2):
                        nc.tensor.matmul(hp[:],
                                         lhsT=wq_sb[dt][:, jt * P:(jt + 1) * P],
                                         rhs=wk_sb[dt][:],
                                         start=(dt == 0), stop=(dt == 1))
                    nc.vector.tensor_copy(h_sb[jt][:], hp[:])
                for it in range(2):
                    for hh in range(2):
                        gp = psetup.tile([P, MBS], F32, tag="g", name=f"gp{it}_{hh}")
                        for jt in range(2):
                            nc.tensor.matmul(
                                gp[:],
                                lhsT=h_sb[jt][:, it * P:(it + 1) * P],
                                rhs=imgT_sb[jt][:, hh * MBS:(hh + 1) * MBS],
                                start=(jt == 0), stop=(jt == 1))
                        nc.vector.tensor_copy(g_sb[it][:, hh * MBS:(hh + 1) * MBS],
                                              gp[:])
            if debug:
                for it in range(2):
                    nc.sync.dma_start(dbg_g[it * P:(it + 1) * P, :], g_sb[it][:])

            # ---- main pools ----
            o_pool = ctx.enter_context(tc.tile_pool(name="opool", bufs=4, space="PSUM"))
            s_pool = ctx.enter_context(tc.tile_pool(name="spool", bufs=PIPE + 1, space="PSUM"))
            rs_pool = ctx.enter_context(tc.tile_pool(name="rspool", bufs=1, space="PSUM"))
            rhs_pool = ctx.enter_context(tc.tile_pool(name="rhs", bufs=12))
            pt_pool = ctx.enter_context(tc.tile_pool(name="pt", bufs=PIPE + 2))
            eout_pool = ctx.enter_context(tc.tile_pool(name="eout", bufs=4))
            rec_pool = ctx.enter_context(tc.tile_pool(name="rec", bufs=4))

            def s_stage(mb, ch):
                """S^T chunk matmuls + exp -> returns PT sbuf tile."""
                g, coff = divmod(ch, TTW // P)
                coff *= P
                sp = s_pool.tile([P, MBS], F32, tag="s", name=f"s{mb}_{ch}")
                for it in range(2):
                    nc.tensor.matmul(
                        sp[:],
                        lhsT=tt_sb[it][g][:, coff:coff + P],
                        rhs=g_sb[it][:, mb * MBS:(mb + 1) * MBS],
                        start=(it == 0), stop=(it == 1))
                pt = pt_pool.tile([P, MBS], F32, tag="pt", name=f"pt{mb}_{ch}")
                nc.scalar.activation(pt[:], sp[:],
                                     mybir.ActivationFunctionType.Exp)
                return pt

            for mb in range(MB):
                o_ps = [o_pool.tile([P, 2 * D], F32, tag="o", name=f"o{mb}_{i}") for i in range(NSUB)]
                rs_ps = rs_pool.tile([P, NSUB], F32, tag="rs", name=f"rs{mb}")

                pts = {}
                for ch in range(PIPE):
                    pts[ch] = s_stage(mb, ch)

                for ch in range(NCH):
                    if ch + PIPE < NCH:
                        pts[ch + PIPE] = s_stage(mb, ch + PIPE)

                    rhs = rhs_pool.tile([P, 2 * D], F32, tag="rhs", name=f"rhs{mb}_{ch}")
                    nc.sync.dma_start(rhs[:, 0:D], img_d[ch * P:(ch + 1) * P, :])
                    nc.sync.dma_start(rhs[:, D:2 * D], text_d[ch * P:(ch + 1) * P, :])

                    pt = pts.pop(ch)
                    if debug and mb == 0 and ch == 0:
                        nc.sync.dma_start(dbg_pt[:, :], pt[:])
                    first, last = (ch == 0), (ch == NCH - 1)
                    for sub in range(NSUB):
                        ptsub = pt[:, sub * P:(sub + 1) * P]
                        nc.tensor.matmul(o_ps[sub][:], lhsT=ptsub, rhs=rhs[:],
                                         start=first, stop=last)
                        # start=True clears the WHOLE psum bank, so only the
                        # first write into the shared rs bank may set it.
                        nc.tensor.matmul(rs_ps[:, sub:sub + 1], lhsT=ptsub,
                                         rhs=ones_sb[:],
                                         start=(first and sub == 0), stop=last,
                                         skip_group_check=True)

                if debug and mb == 0:
                    rs_sb_dbg = eout_pool.tile([P, NSUB], F32, tag="rsdbg", name=f"rsdbg{mb}")
                    nc.vector.tensor_copy(rs_sb_dbg[:], rs_ps[:])
                    nc.sync.dma_start(dbg_rs[:, :], rs_sb_dbg[:])
                for sub in range(NSUB):
                    rec = rec_pool.tile([P, 1], F32, tag="rec", name=f"rec{mb}_{sub}")
                    nc.vector.reciprocal(rec[:], rs_ps[:, sub:sub + 1])
                    osb = eout_pool.tile([P, 2 * D], F32, tag="eout", name=f"eout{mb}_{sub}")
                    nc.vector.tensor_scalar(
                        osb[:], o_ps[sub][:], rec[:], NORM,
                        op0=mybir.AluOpType.mult, op1=mybir.AluOpType.mult)
                    row0 = mb * MBS + sub * P
                    nc.sync.dma_start(out_d[row0:row0 + P, :], osb[:])

    nc.compile()
    return nc


def kernel(img, text, Wq, Wk):
    img = np.ascontiguousarray(img, dtype=np.float32)
    text = np.ascontiguousarray(text, dtype=np.float32)
    Wq = np.ascontiguousarray(Wq, dtype=np.float32)
    Wk = np.ascontiguousarray(Wk, dtype=np.float32)

    if "nc" not in _CACHE:
        _CACHE["nc"] = _build_nc()
    nc = _CACHE["nc"]

    textT = np.ascontiguousarray(text.T)
    in_maps = []
    for c in range(N_CORES):
        in_maps.append({
            "img": img,
            "text": text,
            "textT": textT,
            "imgT_slab": np.ascontiguousarray(img[c * SLAB:(c + 1) * SLAB].T),
            "Wq": Wq,
            "Wk": Wk,
        })

    res = run_bass_kernel_spmd(nc, in_maps, core_ids=list(range(N_CORES)),
                               **_CACHE.get("run_kwargs", {}))
    _CACHE["last_results"] = res
    out = np.concatenate([res.results[c]["out"] for c in range(N_CORES)], axis=0)
    return np.ascontiguousarray(out[:, :D]), np.ascontiguousarray(out[:, D:])


if __name__ == "__main__":
    rng = np.random.default_rng(0)
    img = rng.standard_normal((N, D), dtype=np.float32)
    text = rng.standard_normal((N, D), dtype=np.float32)
    sc = 1.0 / np.sqrt(D)
    Wq = rng.uniform(-sc, sc, (D, D)).astype(np.float32)
    Wk = rng.uniform(-sc, sc, (D, D)).astype(np.float32)
    oi, ot = kernel(img, text, Wq, Wk)
    print("out_img", oi.shape, oi.dtype, "out_text", ot.shape, ot.dtype)


# revision 6
# speedup vs baseline: 3.2018x; 3.2018x over previous
"""Joint attention layer on 8 trn2 NeuronCores (query-sharded, SPMD).

Math (reference):
    Q = img @ Wq.T ; K = text @ Wk.T ; S = Q @ K.T        [N, N]
    attn = softmax(S, axis=1) / sqrt(D)
    out_img = attn @ img ; out_text = attn @ text

Per-core plan (core c owns query rows m in [c*1024, (c+1)*1024)):
    H[j,i]  = sum_d Wq[d,j] Wk[d,i]              (= Wq.T @ Wk, 256x256)
    G[i,m]  = sum_j H[j,i] imgT[j,m]             (absorbs both projections)
    S^T[n,m] = sum_i text[n,i] G[i,m]            (keys on partitions)
    P^T = exp(S^T)  (no max subtraction needed: |S| <~ 55 << 88)
    O[m,:] = sum_n P^T[n,m] * [img|text][n,:]    (PSUM accum over all n)
    rowsum[m] = sum_n P^T[n,m]                   (ones-lhsT matmul, [1,512])
    out[m,:] = O[m,:] / rowsum[m] / sqrt(D)

Precision: S-chain (Wq,Wk,H,imgT,G,textT) in fp16 (values are O(1));
P^T and the O matmul in bf16 (exp values reach ~e^55, beyond fp16 range);
all accumulation in fp32 PSUM; epilogue in fp32.  fp32 matmuls are ~6x
slower per column on the PE, so they are avoided everywhere.

Host passes img/text both natural (bf16) and transposed (fp16) so the
kernel never transposes on device. No collectives: outputs are disjoint
row slabs concatenated on the host.
"""

import numpy as np
import ml_dtypes
from contextlib import ExitStack

import concourse.bass as bass
import concourse.tile as tile
from concourse import bacc, mybir
from concourse.bass_utils import run_bass_kernel_spmd

F32 = mybir.dt.float32
F16 = mybir.dt.float16
BF16 = mybir.dt.bfloat16
P = 128          # partitions
D = 256          # hidden dim
N = 8192         # sequence length
N_CORES = 8
SLAB = N // N_CORES          # 1024 query rows per core
MB = 2                       # m-blocks per core
MBS = SLAB // MB             # 512 queries per m-block
NSUB = MBS // P              # 4 psum subtiles per m-block
NCH = N // P                 # 64 key chunks of 128
TTG = 8                      # textT column-group tiles
TTW = N // TTG               # 1024 cols per group
PIPE = 2                     # S-stage lookahead (chunks)
NORM = 1.0 / 16.0            # 1/sqrt(D)

_CACHE = {}


def _build_nc(debug=False):
    nc = bacc.Bacc("TRN2", target_bir_lowering=False, debug=False,
                   num_devices=N_CORES)

    imgb_d = nc.dram_tensor("img_bf16", [N, D], BF16, kind="ExternalInput").ap()
    textb_d = nc.dram_tensor("text_bf16", [N, D], BF16, kind="ExternalInput").ap()
    textT_d = nc.dram_tensor("textT_f16", [D, N], F16, kind="ExternalInput").ap()
    imgT_d = nc.dram_tensor("imgT_f16", [D, SLAB], F16, kind="ExternalInput").ap()
    wq_d = nc.dram_tensor("Wq_f16", [D, D], F16, kind="ExternalInput").ap()
    wk_d = nc.dram_tensor("Wk_f16", [D, D], F16, kind="ExternalInput").ap()
    out_d = nc.dram_tensor("out", [SLAB, 2 * D], F32, kind="ExternalOutput").ap()
    if debug:
        dbg_g = nc.dram_tensor("dbg_g", [D, SLAB], F32, kind="ExternalOutput").ap()
        dbg_pt = nc.dram_tensor("dbg_pt", [P, MBS], F32, kind="ExternalOutput").ap()
        dbg_rs = nc.dram_tensor("dbg_rs", [P, NSUB], F32, kind="ExternalOutput").ap()

    with tile.TileContext(nc) as tc:
        with ExitStack() as ctx:
            const = ctx.enter_context(tc.tile_pool(name="const", bufs=1))

            wq_sb = [const.tile([P, D], F16, name=f"wq{t}") for t in range(2)]
            wk_sb = [const.tile([P, D], F16, name=f"wk{t}") for t in range(2)]
            imgT_sb = [const.tile([P, SLAB], F16, name=f"imgT{t}") for t in range(2)]
            for t in range(2):
                nc.sync.dma_start(wq_sb[t][:], wq_d[t * P:(t + 1) * P, :])
                nc.sync.dma_start(wk_sb[t][:], wk_d[t * P:(t + 1) * P, :])
                nc.sync.dma_start(imgT_sb[t][:], imgT_d[t * P:(t + 1) * P, :])

            tt_sb = [[const.tile([P, TTW], F16, name=f"tt{it}_{g}")
                      for g in range(TTG)] for it in range(2)]
            for it in range(2):
                for g in range(TTG):
                    nc.sync.dma_start(
                        tt_sb[it][g][:],
                        textT_d[it * P:(it + 1) * P, g * TTW:(g + 1) * TTW])

            ones_sb = const.tile([P, 1], BF16, name="ones")
            nc.vector.memset(ones_sb[:], 1.0)
            one_f32 = const.tile([1, 1], F32, name="one_f32")
            nc.vector.memset(one_f32[:], 1.0)

            h_sb = [const.tile([P, D], F16, name=f"h{jt}") for jt in range(2)]
            g_sb = [const.tile([P, SLAB], F16, name=f"g{it}") for it in range(2)]

            # ---- setup: H = Wq.T @ Wk ; G[i,m] = sum_j H[j,i] imgT[j,m] ----
            with tc.tile_pool(name="psetup", bufs=2, space="PSUM") as psetup:
                for jt in range(2):
                    hp = psetup.tile([P, D], F32, tag="h", name=f"hp{jt}")
                    for dt in range(2):
                        nc.tensor.matmul(hp[:],
                                         lhsT=wq_sb[dt][:, jt * P:(jt + 1) * P],
                                         rhs=wk_sb[dt][:],
                                         start=(dt == 0), stop=(dt == 1))
                    nc.vector.tensor_copy(h_sb[jt][:], hp[:])
                for it in range(2):
                    for hh in range(2):
                        gp = psetup.tile([P, MBS], F32, tag="g", name=f"gp{it}_{hh}")
                        for jt in range(2):
                            nc.tensor.matmul(
                                gp[:],
                                lhsT=h_sb[jt][:, it * P:(it + 1) * P],
                                rhs=imgT_sb[jt][:, hh * MBS:(hh + 1) * MBS],
                                start=(jt == 0), stop=(jt == 1))
                        nc.vector.tensor_copy(g_sb[it][:, hh * MBS:(hh + 1) * MBS],
                                              gp[:])
            if debug:
                for it in range(2):
                    dbg_g_sb = const.tile([P, SLAB], F32, name=f"dbg_g_sb{it}")
                    nc.vector.tensor_copy(dbg_g_sb[:], g_sb[it][:])
                    nc.sync.dma_start(dbg_g[it * P:(it + 1) * P, :], dbg_g_sb[:])

            # ---- main pools ----
            o_pool = ctx.enter_context(tc.tile_pool(name="opool", bufs=4, space="PSUM"))
            s_pool = ctx.enter_context(tc.tile_pool(name="spool", bufs=PIPE + 1, space="PSUM"))
            rs_pool = ctx.enter_context(tc.tile_pool(name="rspool", bufs=1, space="PSUM"))
            rhs_pool = ctx.enter_context(tc.tile_pool(name="rhs", bufs=NCH))
            pt_pool = ctx.enter_context(tc.tile_pool(name="pt", bufs=PIPE + 2))
            eout_pool = ctx.enter_context(tc.tile_pool(name="eout", bufs=4))
            rec_pool = ctx.enter_context(tc.tile_pool(name="rec", bufs=2 * MB))

            rhs_tiles = {}

            def s_stage(mb, ch):
                """S^T chunk matmuls + exp -> returns PT sbuf tile (bf16)."""
                g, coff = divmod(ch, TTW // P)
                coff *= P
                sp = s_pool.tile([P, MBS], F32, tag="s", name=f"s{mb}_{ch}")
                for it in range(2):
                    nc.tensor.matmul(
                        sp[:],
                        lhsT=tt_sb[it][g][:, coff:coff + P],
                        rhs=g_sb[it][:, mb * MBS:(mb + 1) * MBS],
                        start=(it == 0), stop=(it == 1))
                pt = pt_pool.tile([P, MBS], BF16, tag="pt", name=f"pt{mb}_{ch}")
                nc.scalar.activation(pt[:], sp[:],
                                     mybir.ActivationFunctionType.Exp)
                return pt

            for mb in range(MB):
                o_ps = [o_pool.tile([P, 2 * D], F32, tag="o", name=f"o{mb}_{i}")
                        for i in range(NSUB)]
                rs_ps = rs_pool.tile([1, MBS], F32, tag="rs", name=f"rs{mb}")

                pts = {}
                for ch in range(PIPE):
                    pts[ch] = s_stage(mb, ch)

                for ch in range(NCH):
                    if ch + PIPE < NCH:
                        pts[ch + PIPE] = s_stage(mb, ch + PIPE)

                    if mb == 0:
                        rhs = rhs_pool.tile([P, 2 * D], BF16, tag="rhs",
                                            name=f"rhs{ch}")
                        nc.sync.dma_start(rhs[:, 0:D],
                                          imgb_d[ch * P:(ch + 1) * P, :])
                        nc.sync.dma_start(rhs[:, D:2 * D],
                                          textb_d[ch * P:(ch + 1) * P, :])
                        rhs_tiles[ch] = rhs
                    else:
                        rhs = rhs_tiles[ch]

                    pt = pts.pop(ch)
                    if debug and mb == 0 and ch == 0:
                        dbg_pt_sb = const.tile([P, MBS], F32, name="dbg_pt_sb")
                        nc.vector.tensor_copy(dbg_pt_sb[:], pt[:])
                        nc.sync.dma_start(dbg_pt[:, :], dbg_pt_sb[:])
                    first, last = (ch == 0), (ch == NCH - 1)
                    for sub in range(NSUB):
                        nc.tensor.matmul(o_ps[sub][:],
                                         lhsT=pt[:, sub * P:(sub + 1) * P],
                                         rhs=rhs[:], start=first, stop=last)
                    nc.tensor.matmul(rs_ps[:], lhsT=ones_sb[:], rhs=pt[:],
                                     start=first, stop=last)

                # ---- epilogue: rowsum row -> per-partition recips ----
                rs_sb = rec_pool.tile([1, MBS], F32, tag="rssb", name=f"rssb{mb}")
                nc.vector.tensor_copy(rs_sb[:], rs_ps[:])
                # transpose [1,512] -> [128,4] via K=1 matmuls against a 1x1
                # ones tile (start=True clears the whole bank: only sub 0 sets it)
                tr_ps = s_pool.tile([P, NSUB], F32, tag="s", name=f"tr{mb}")
                for sub in range(NSUB):
                    nc.tensor.matmul(tr_ps[:, sub:sub + 1],
                                     lhsT=rs_sb[0:1, sub * P:(sub + 1) * P],
                                     rhs=one_f32[:],
                                     start=(sub == 0), stop=(sub == NSUB - 1),
                                     skip_group_check=True)
                recip = rec_pool.tile([P, NSUB], F32, tag="recip",
                                      name=f"recip{mb}")
                nc.vector.reciprocal(recip[:], tr_ps[:])
                if debug and mb == 0:
                    rec_dbg = rec_pool.tile([P, NSUB], F32, tag="recdbg", name="recdbg")
                    nc.vector.tensor_copy(rec_dbg[:], tr_ps[:])
                    nc.sync.dma_start(dbg_rs[:, :], rec_dbg[:])
                for sub in range(NSUB):
                    osb = eout_pool.tile([P, 2 * D], F32, tag="eout",
                                         name=f"eout{mb}_{sub}")
                    nc.vector.tensor_scalar(
                        osb[:], o_ps[sub][:], recip[:, sub:sub + 1], NORM,
                        op0=mybir.AluOpType.mult, op1=mybir.AluOpType.mult)
                    row0 = mb * MBS + sub * P
                    nc.sync.dma_start(out_d[row0:row0 + P, :], osb[:])

    nc.compile()
    return nc


def kernel(img, text, Wq, Wk):
    img = np.ascontiguousarray(img, dtype=np.float32)
    text = np.ascontiguousarray(text, dtype=np.float32)

    if "nc" not in _CACHE:
        _CACHE["nc"] = _build_nc()
    nc = _CACHE["nc"]

    textT16 = np.ascontiguousarray(text.T.astype(np.float16))
    img_bf = img.astype(ml_dtypes.bfloat16)
    text_bf = text.astype(ml_dtypes.bfloat16)
    wq16 = np.asarray(Wq, dtype=np.float16)
    wk16 = np.asarray(Wk, dtype=np.float16)

    in_maps = []
    for c in range(N_CORES):
        in_maps.append({
            "img_bf16": img_bf,
            "text_bf16": text_bf,
            "textT_f16": textT16,
            "imgT_f16": np.ascontiguousarray(
                img[c * SLAB:(c + 1) * SLAB].T.astype(np.float16)),
            "Wq_f16": wq16,
            "Wk_f16": wk16,
        })

    res = run_bass_kernel_spmd(nc, in_maps, core_ids=list(range(N_CORES)),
                               **_CACHE.get("run_kwargs", {}))
    _CACHE["last_results"] = res
    out = np.concatenate([res.results[c]["out"] for c in range(N_CORES)], axis=0)
    return np.ascontiguousarray(out[:, :D]), np.ascontiguousarray(out[:, D:])


if __name__ == "__main__":
    rng = np.random.default_rng(0)
    img = rng.standard_normal((N, D), dtype=np.float32)
    text = rng.standard_normal((N, D), dtype=np.float32)
    sc = 1.0 / np.sqrt(D)
    Wq = rng.uniform(-sc, sc, (D, D)).astype(np.float32)
    Wk = rng.uniform(-sc, sc, (D, D)).astype(np.float32)
    oi, ot = kernel(img, text, Wq, Wk)
    print("out_img", oi.shape, oi.dtype, "out_text", ot.shape, ot.dtype)


# revision 7
# speedup vs baseline: 3.2111x; 1.0029x over previous
"""Joint attention layer on 8 trn2 NeuronCores (query-sharded, SPMD).

Math (reference):
    Q = img @ Wq.T ; K = text @ Wk.T ; S = Q @ K.T        [N, N]
    attn = softmax(S, axis=1) / sqrt(D)
    out_img = attn @ img ; out_text = attn @ text

Per-core plan (core c owns query rows m in [c*1024, (c+1)*1024)):
    H[j,i]  = sum_d Wq[d,j] Wk[d,i]              (= Wq.T @ Wk, 256x256)
    G[i,m]  = sum_j H[j,i] imgT[j,m]             (absorbs both projections)
    S^T[n,m] = sum_i text[n,i] G[i,m]            (keys on partitions)
    P^T = exp(S^T)  (no max subtraction needed: |S| <~ 55 << 88)
    O[m,:] = sum_n P^T[n,m] * [img|text][n,:]    (PSUM accum over all n)
    rowsum[m] = sum_n P^T[n,m]                   (ones-lhsT matmul, [1,512])
    out[m,:] = O[m,:] / rowsum[m] / sqrt(D)

Precision: S-chain (Wq,Wk,H,imgT,G,textT) in fp16 (values are O(1));
P^T and the O matmul in bf16 (exp values reach ~e^55, beyond fp16 range);
all accumulation in fp32 PSUM; epilogue in fp32.  fp32 matmuls are ~6x
slower per column on the PE, so they are avoided everywhere.

Host passes img/text both natural (bf16) and transposed (fp16) so the
kernel never transposes on device. No collectives: outputs are disjoint
row slabs concatenated on the host.
"""

import numpy as np
import ml_dtypes
from contextlib import ExitStack

import concourse.bass as bass
import concourse.tile as tile
from concourse import bacc, mybir
from concourse.bass_utils import run_bass_kernel_spmd

F32 = mybir.dt.float32
F16 = mybir.dt.float16
BF16 = mybir.dt.bfloat16
P = 128          # partitions
D = 256          # hidden dim
N = 8192         # sequence length
N_CORES = 8
SLAB = N // N_CORES          # 1024 query rows per core
MB = 2                       # m-blocks per core
MBS = SLAB // MB             # 512 queries per m-block
NSUB = MBS // P              # 4 psum subtiles per m-block
NCH = N // P                 # 64 key chunks of 128
TTG = 8                      # textT column-group tiles
TTW = N // TTG               # 1024 cols per group
PIPE = 2                     # S-stage lookahead (chunks)
NORM = 1.0 / 16.0            # 1/sqrt(D)

_CACHE = {}


def _build_nc(debug=False):
    nc = bacc.Bacc("TRN2", target_bir_lowering=False, debug=False,
                   num_devices=N_CORES)

    imgb_d = nc.dram_tensor("img_bf16", [N, D], BF16, kind="ExternalInput").ap()
    textb_d = nc.dram_tensor("text_bf16", [N, D], BF16, kind="ExternalInput").ap()
    textT_d = nc.dram_tensor("textT_f16", [D, N], F16, kind="ExternalInput").ap()
    imgT_d = nc.dram_tensor("imgT_f16", [D, SLAB], F16, kind="ExternalInput").ap()
    wq_d = nc.dram_tensor("Wq_f16", [D, D], F16, kind="ExternalInput").ap()
    wk_d = nc.dram_tensor("Wk_f16", [D, D], F16, kind="ExternalInput").ap()
    out_d = nc.dram_tensor("out", [SLAB, 2 * D], F32, kind="ExternalOutput").ap()
    if debug:
        dbg_g = nc.dram_tensor("dbg_g", [D, SLAB], F32, kind="ExternalOutput").ap()
        dbg_pt = nc.dram_tensor("dbg_pt", [P, MBS], F32, kind="ExternalOutput").ap()
        dbg_rs = nc.dram_tensor("dbg_rs", [P, NSUB], F32, kind="ExternalOutput").ap()

    with tile.TileContext(nc) as tc:
        with ExitStack() as ctx:
            const = ctx.enter_context(tc.tile_pool(name="const", bufs=1))

            wq_sb = [const.tile([P, D], F16, name=f"wq{t}") for t in range(2)]
            wk_sb = [const.tile([P, D], F16, name=f"wk{t}") for t in range(2)]
            imgT_sb = [const.tile([P, SLAB], F16, name=f"imgT{t}") for t in range(2)]
            for t in range(2):
                nc.sync.dma_start(wq_sb[t][:], wq_d[t * P:(t + 1) * P, :])
                nc.sync.dma_start(wk_sb[t][:], wk_d[t * P:(t + 1) * P, :])
                nc.sync.dma_start(imgT_sb[t][:], imgT_d[t * P:(t + 1) * P, :])

            tt_sb = [[const.tile([P, TTW], F16, name=f"tt{it}_{g}")
                      for g in range(TTG)] for it in range(2)]
            for it in range(2):
                for g in range(TTG):
                    nc.sync.dma_start(
                        tt_sb[it][g][:],
                        textT_d[it * P:(it + 1) * P, g * TTW:(g + 1) * TTW])

            ones_sb = const.tile([P, 1], BF16, name="ones")
            nc.vector.memset(ones_sb[:], 1.0)
            one_f32 = const.tile([1, 1], F32, name="one_f32")
            nc.vector.memset(one_f32[:], 1.0)

            h_sb = [const.tile([P, D], F16, name=f"h{jt}") for jt in range(2)]
            g_sb = [const.tile([P, SLAB], F16, name=f"g{it}") for it in range(2)]

            # ---- setup: H = Wq.T @ Wk ; G[i,m] = sum_j H[j,i] imgT[j,m] ----
            with tc.tile_pool(name="psetup", bufs=2, space="PSUM") as psetup:
                for jt in range(2):
                    hp = psetup.tile([P, D], F32, tag="h", name=f"hp{jt}")
                    for dt in range(2):
                        nc.tensor.matmul(hp[:],
                                         lhsT=wq_sb[dt][:, jt * P:(jt + 1) * P],
                                         rhs=wk_sb[dt][:],
                                         start=(dt == 0), stop=(dt == 1))
                    nc.vector.tensor_copy(h_sb[jt][:], hp[:])
                for it in range(2):
                    for hh in range(2):
                        gp = psetup.tile([P, MBS], F32, tag="g", name=f"gp{it}_{hh}")
                        for jt in range(2):
                            nc.tensor.matmul(
                                gp[:],
                                lhsT=h_sb[jt][:, it * P:(it + 1) * P],
                                rhs=imgT_sb[jt][:, hh * MBS:(hh + 1) * MBS],
                                start=(jt == 0), stop=(jt == 1))
                        nc.vector.tensor_copy(g_sb[it][:, hh * MBS:(hh + 1) * MBS],
                                              gp[:])
            if debug:
                for it in range(2):
                    dbg_g_sb = const.tile([P, SLAB], F32, name=f"dbg_g_sb{it}")
                    nc.vector.tensor_copy(dbg_g_sb[:], g_sb[it][:])
                    nc.sync.dma_start(dbg_g[it * P:(it + 1) * P, :], dbg_g_sb[:])

            # ---- main pools ----
            o_pool = ctx.enter_context(tc.tile_pool(name="opool", bufs=4, space="PSUM"))
            s_pool = ctx.enter_context(tc.tile_pool(name="spool", bufs=PIPE + 1, space="PSUM"))
            rs_pool = ctx.enter_context(tc.tile_pool(name="rspool", bufs=1, space="PSUM"))
            rhs_pool = ctx.enter_context(tc.tile_pool(name="rhs", bufs=NCH))
            pt_pool = ctx.enter_context(tc.tile_pool(name="pt", bufs=PIPE + 2))
            eout_pool = ctx.enter_context(tc.tile_pool(name="eout", bufs=4))
            rec_pool = ctx.enter_context(tc.tile_pool(name="rec", bufs=2 * MB))

            rhs_tiles = {}

            def s_mm(mb, ch, it, sp):
                g, coff = divmod(ch, TTW // P)
                coff *= P
                nc.tensor.matmul(
                    sp[:],
                    lhsT=tt_sb[it][g][:, coff:coff + P],
                    rhs=g_sb[it][:, mb * MBS:(mb + 1) * MBS],
                    start=(it == 0), stop=(it == 1))

            def s_act(mb, ch, sp):
                pt = pt_pool.tile([P, MBS], BF16, tag="pt", name=f"pt{mb}_{ch}")
                nc.scalar.activation(pt[:], sp[:],
                                     mybir.ActivationFunctionType.Exp)
                return pt

            for mb in range(MB):
                o_ps = [o_pool.tile([P, 2 * D], F32, tag="o", name=f"o{mb}_{i}")
                        for i in range(NSUB)]
                rs_ps = rs_pool.tile([1, MBS], F32, tag="rs", name=f"rs{mb}")

                pts = {}
                for ch in range(PIPE):
                    sp = s_pool.tile([P, MBS], F32, tag="s", name=f"s{mb}_{ch}")
                    s_mm(mb, ch, 0, sp)
                    s_mm(mb, ch, 1, sp)
                    pts[ch] = s_act(mb, ch, sp)

                for ch in range(NCH):
                    nxt = ch + PIPE
                    sp_n = None
                    if nxt < NCH:
                        sp_n = s_pool.tile([P, MBS], F32, tag="s",
                                           name=f"s{mb}_{nxt}")

                    if mb == 0:
                        rhs = rhs_pool.tile([P, 2 * D], BF16, tag="rhs",
                                            name=f"rhs{ch}")
                        nc.sync.dma_start(rhs[:, 0:D],
                                          imgb_d[ch * P:(ch + 1) * P, :])
                        nc.sync.dma_start(rhs[:, D:2 * D],
                                          textb_d[ch * P:(ch + 1) * P, :])
                        rhs_tiles[ch] = rhs
                    else:
                        rhs = rhs_tiles[ch]

                    pt = pts.pop(ch)
                    if debug and mb == 0 and ch == 0:
                        dbg_pt_sb = const.tile([P, MBS], F32, name="dbg_pt_sb")
                        nc.vector.tensor_copy(dbg_pt_sb[:], pt[:])
                        nc.sync.dma_start(dbg_pt[:, :], dbg_pt_sb[:])
                    first, last = (ch == 0), (ch == NCH - 1)

                    def o_mm(sub):
                        nc.tensor.matmul(o_ps[sub][:],
                                         lhsT=pt[:, sub * P:(sub + 1) * P],
                                         rhs=rhs[:], start=first, stop=last)

                    # Interleave fresh-weight MMs (S, rs) between pt-weight O
                    # MMs so every LDWEIGHTS hides under a full 512-col stream.
                    if sp_n is not None:
                        s_mm(mb, nxt, 0, sp_n)
                    o_mm(0)
                    if sp_n is not None:
                        s_mm(mb, nxt, 1, sp_n)
                        pts[nxt] = s_act(mb, nxt, sp_n)
                    o_mm(1)
                    nc.tensor.matmul(rs_ps[:], lhsT=ones_sb[:], rhs=pt[:],
                                     start=first, stop=last)
                    o_mm(2)
                    o_mm(3)

                # ---- epilogue: rowsum row -> per-partition recips ----
                rs_sb = rec_pool.tile([1, MBS], F32, tag="rssb", name=f"rssb{mb}")
                nc.vector.tensor_copy(rs_sb[:], rs_ps[:])
                # transpose [1,512] -> [128,4] via K=1 matmuls against a 1x1
                # ones tile (start=True clears the whole bank: only sub 0 sets it)
                tr_ps = s_pool.tile([P, NSUB], F32, tag="s", name=f"tr{mb}")
                for sub in range(NSUB):
                    nc.tensor.matmul(tr_ps[:, sub:sub + 1],
                                     lhsT=rs_sb[0:1, sub * P:(sub + 1) * P],
                                     rhs=one_f32[:],
                                     start=(sub == 0), stop=(sub == NSUB - 1),
                                     skip_group_check=True)
                recip = rec_pool.tile([P, NSUB], F32, tag="recip",
                                      name=f"recip{mb}")
                nc.vector.reciprocal(recip[:], tr_ps[:])
                if debug and mb == 0:
                    rec_dbg = rec_pool.tile([P, NSUB], F32, tag="recdbg", name="recdbg")
                    nc.vector.tensor_copy(rec_dbg[:], tr_ps[:])
                    nc.sync.dma_start(dbg_rs[:, :], rec_dbg[:])
                for sub in range(NSUB):
                    osb = eout_pool.tile([P, 2 * D], F32, tag="eout",
                                         name=f"eout{mb}_{sub}")
                    nc.vector.tensor_scalar(
                        osb[:], o_ps[sub][:], recip[:, sub:sub + 1], NORM,
                        op0=mybir.AluOpType.mult, op1=mybir.AluOpType.mult)
                    row0 = mb * MBS + sub * P
                    nc.sync.dma_start(out_d[row0:row0 + P, :], osb[:])

    nc.compile()
    return nc


def kernel(img, text, Wq, Wk):
    img = np.ascontiguousarray(img, dtype=np.float32)
    text = np.ascontiguousarray(text, dtype=np.float32)

    if "nc" not in _CACHE:
        _CACHE["nc"] = _build_nc()
    nc = _CACHE["nc"]

    textT16 = np.ascontiguousarray(text.T.astype(np.float16))
    img_bf = img.astype(ml_dtypes.bfloat16)
    text_bf = text.astype(ml_dtypes.bfloat16)
    wq16 = np.asarray(Wq, dtype=np.float16)
    wk16 = np.asarray(Wk, dtype=np.float16)

    in_maps = []
    for c in range(N_CORES):
        in_maps.append({
            "img_bf16": img_bf,
            "text_bf16": text_bf,
            "textT_f16": textT16,
            "imgT_f16": np.ascontiguousarray(
                img[c * SLAB:(c + 1) * SLAB].T.astype(np.float16)),
            "Wq_f16": wq16,
            "Wk_f16": wk16,
        })

    res = run_bass_kernel_spmd(nc, in_maps, core_ids=list(range(N_CORES)),
                               **_CACHE.get("run_kwargs", {}))
    _CACHE["last_results"] = res
    out = np.concatenate([res.results[c]["out"] for c in range(N_CORES)], axis=0)
    return np.ascontiguousarray(out[:, :D]), np.ascontiguousarray(out[:, D:])


if __name__ == "__main__":
    rng = np.random.default_rng(0)
    img = rng.standard_normal((N, D), dtype=np.float32)
    text = rng.standard_normal((N, D), dtype=np.float32)
    sc = 1.0 / np.sqrt(D)
    Wq = rng.uniform(-sc, sc, (D, D)).astype(np.float32)
    Wk = rng.uniform(-sc, sc, (D, D)).astype(np.float32)
    oi, ot = kernel(img, text, Wq, Wk)
    print("out_img", oi.shape, oi.dtype, "out_text", ot.shape, ot.dtype)


# revision 9
# speedup vs baseline: 3.3994x; 1.0586x over previous
"""Joint attention layer on 8 trn2 NeuronCores (query-sharded, SPMD).

Math (reference):
    Q = img @ Wq.T ; K = text @ Wk.T ; S = Q @ K.T        [N, N]
    attn = softmax(S, axis=1) / sqrt(D)
    out_img = attn @ img ; out_text = attn @ text

Per-core plan (core c owns query rows m in [c*1024, (c+1)*1024)):
    H[j,i]  = sum_d Wq[d,j] Wk[d,i]              (= Wq.T @ Wk, 256x256)
    G[i,m]  = sum_j H[j,i] imgT[j,m]             (absorbs both projections)
    S^T[n,m] = sum_i text[n,i] G[i,m]            (keys on partitions)
    P^T = exp(S^T)  (no max subtraction needed: |S| <~ 55 << 88)
    O[m,:] = sum_n P^T[n,m] * [img|text][n,:]    (PSUM accum over all n)
    rowsum[m] = sum_n P^T[n,m]                   (ones-lhsT matmul, [1,512])
    out[m,:] = O[m,:] / rowsum[m] / sqrt(D)

Precision: S-chain (Wq,Wk,H,imgT,G,textT) in fp16 (values are O(1));
P^T and the O matmul in bf16 (exp values reach ~e^55, beyond fp16 range);
all accumulation in fp32 PSUM; epilogue in fp32.  fp32 matmuls are ~6x
slower per column on the PE, so they are avoided everywhere.

Host passes img/text both natural (bf16) and transposed (fp16) so the
kernel never transposes on device. No collectives: outputs are disjoint
row slabs concatenated on the host.
"""

import numpy as np
import ml_dtypes
from contextlib import ExitStack

import concourse.bass as bass
import concourse.tile as tile
from concourse import bacc, mybir
from concourse.bass_utils import run_bass_kernel_spmd

F32 = mybir.dt.float32
F16 = mybir.dt.float16
BF16 = mybir.dt.bfloat16
P = 128          # partitions
D = 256          # hidden dim
N = 8192         # sequence length
N_CORES = 8
SLAB = N // N_CORES          # 1024 query rows per core
MB = 2                       # m-blocks per core
MBS = SLAB // MB             # 512 queries per m-block
NSUB = MBS // P              # 4 psum subtiles per m-block
NCH = N // P                 # 64 key chunks of 128
TTG = 8                      # textT column-group tiles
TTW = N // TTG               # 1024 cols per group
PIPE = 2                     # S-stage lookahead (chunks)
RS_GROUP = 4                 # rowsum MMs batched to amortize array reconfig
NORM = 1.0 / 16.0            # 1/sqrt(D)

_CACHE = {}


def _build_nc(debug=False):
    nc = bacc.Bacc("TRN2", target_bir_lowering=False, debug=False,
                   num_devices=N_CORES)

    imgb_d = nc.dram_tensor("img_bf16", [N, D], BF16, kind="ExternalInput").ap()
    textb_d = nc.dram_tensor("text_bf16", [N, D], BF16, kind="ExternalInput").ap()
    textT_d = nc.dram_tensor("textT_f16", [D, N], F16, kind="ExternalInput").ap()
    imgT_d = nc.dram_tensor("imgT_f16", [D, SLAB], F16, kind="ExternalInput").ap()
    wq_d = nc.dram_tensor("Wq_f16", [D, D], F16, kind="ExternalInput").ap()
    wk_d = nc.dram_tensor("Wk_f16", [D, D], F16, kind="ExternalInput").ap()
    out_d = nc.dram_tensor("out", [SLAB, 2 * D], F32, kind="ExternalOutput").ap()
    if debug:
        dbg_g = nc.dram_tensor("dbg_g", [D, SLAB], F32, kind="ExternalOutput").ap()
        dbg_pt = nc.dram_tensor("dbg_pt", [P, MBS], F32, kind="ExternalOutput").ap()
        dbg_rs = nc.dram_tensor("dbg_rs", [P, NSUB], F32, kind="ExternalOutput").ap()

    with tile.TileContext(nc) as tc:
        with ExitStack() as ctx:
            const = ctx.enter_context(tc.tile_pool(name="const", bufs=1))

            wq_sb = [const.tile([P, D], F16, name=f"wq{t}") for t in range(2)]
            wk_sb = [const.tile([P, D], F16, name=f"wk{t}") for t in range(2)]
            imgT_sb = [const.tile([P, SLAB], F16, name=f"imgT{t}") for t in range(2)]
            for t in range(2):
                nc.sync.dma_start(wq_sb[t][:], wq_d[t * P:(t + 1) * P, :])
                nc.sync.dma_start(wk_sb[t][:], wk_d[t * P:(t + 1) * P, :])
                nc.sync.dma_start(imgT_sb[t][:], imgT_d[t * P:(t + 1) * P, :])

            tt_sb = [[const.tile([P, TTW], F16, name=f"tt{it}_{g}")
                      for g in range(TTG)] for it in range(2)]
            for it in range(2):
                for g in range(TTG):
                    nc.sync.dma_start(
                        tt_sb[it][g][:],
                        textT_d[it * P:(it + 1) * P, g * TTW:(g + 1) * TTW])

            ones_sb = const.tile([P, 1], BF16, name="ones")
            nc.vector.memset(ones_sb[:], 1.0)
            one_f32 = const.tile([1, 1], F32, name="one_f32")
            nc.vector.memset(one_f32[:], 1.0)

            h_sb = [const.tile([P, D], F16, name=f"h{jt}") for jt in range(2)]
            g_sb = [const.tile([P, SLAB], F16, name=f"g{it}") for it in range(2)]

            # ---- setup: H = Wq.T @ Wk ; G[i,m] = sum_j H[j,i] imgT[j,m] ----
            with tc.tile_pool(name="psetup", bufs=2, space="PSUM") as psetup:
                for jt in range(2):
                    hp = psetup.tile([P, D], F32, tag="h", name=f"hp{jt}")
                    for dt in range(2):
                        nc.tensor.matmul(hp[:],
                                         lhsT=wq_sb[dt][:, jt * P:(jt + 1) * P],
                                         rhs=wk_sb[dt][:],
                                         start=(dt == 0), stop=(dt == 1))
                    nc.vector.tensor_copy(h_sb[jt][:], hp[:])
                for it in range(2):
                    for hh in range(2):
                        gp = psetup.tile([P, MBS], F32, tag="g", name=f"gp{it}_{hh}")
                        for jt in range(2):
                            nc.tensor.matmul(
                                gp[:],
                                lhsT=h_sb[jt][:, it * P:(it + 1) * P],
                                rhs=imgT_sb[jt][:, hh * MBS:(hh + 1) * MBS],
                                start=(jt == 0), stop=(jt == 1))
                        nc.vector.tensor_copy(g_sb[it][:, hh * MBS:(hh + 1) * MBS],
                                              gp[:])
            if debug:
                for it in range(2):
                    dbg_g_sb = const.tile([P, SLAB], F32, name=f"dbg_g_sb{it}")
                    nc.vector.tensor_copy(dbg_g_sb[:], g_sb[it][:])
                    nc.sync.dma_start(dbg_g[it * P:(it + 1) * P, :], dbg_g_sb[:])

            # ---- main pools ----
            o_pool = ctx.enter_context(tc.tile_pool(name="opool", bufs=4, space="PSUM"))
            s_pool = ctx.enter_context(tc.tile_pool(name="spool", bufs=PIPE + 1, space="PSUM"))
            rs_pool = ctx.enter_context(tc.tile_pool(name="rspool", bufs=1, space="PSUM"))
            rhs_pool = ctx.enter_context(tc.tile_pool(name="rhs", bufs=NCH))
            pt_pool = ctx.enter_context(tc.tile_pool(name="pt", bufs=PIPE + 2 + RS_GROUP))
            eout_pool = ctx.enter_context(tc.tile_pool(name="eout", bufs=4))
            rec_pool = ctx.enter_context(tc.tile_pool(name="rec", bufs=2 * MB))

            rhs_tiles = {}

            def s_mm(mb, ch, it, sp):
                g, coff = divmod(ch, TTW // P)
                coff *= P
                nc.tensor.matmul(
                    sp[:],
                    lhsT=tt_sb[it][g][:, coff:coff + P],
                    rhs=g_sb[it][:, mb * MBS:(mb + 1) * MBS],
                    start=(it == 0), stop=(it == 1))

            def s_act(mb, ch, sp):
                pt = pt_pool.tile([P, MBS], BF16, tag="pt", name=f"pt{mb}_{ch}")
                nc.scalar.activation(pt[:], sp[:],
                                     mybir.ActivationFunctionType.Exp)
                return pt

            for mb in range(MB):
                o_ps = [o_pool.tile([P, 2 * D], F32, tag="o", name=f"o{mb}_{i}")
                        for i in range(NSUB)]
                rs_ps = rs_pool.tile([1, MBS], F32, tag="rs", name=f"rs{mb}")

                pts = {}
                pending_rs = []
                for ch in range(PIPE):
                    sp = s_pool.tile([P, MBS], F32, tag="s", name=f"s{mb}_{ch}")
                    s_mm(mb, ch, 0, sp)
                    s_mm(mb, ch, 1, sp)
                    pts[ch] = s_act(mb, ch, sp)

                for ch in range(NCH):
                    nxt = ch + PIPE
                    sp_n = None
                    if nxt < NCH:
                        sp_n = s_pool.tile([P, MBS], F32, tag="s",
                                           name=f"s{mb}_{nxt}")

                    if mb == 0:
                        rhs = rhs_pool.tile([P, 2 * D], BF16, tag="rhs",
                                            name=f"rhs{ch}")
                        nc.sync.dma_start(rhs[:, 0:D],
                                          imgb_d[ch * P:(ch + 1) * P, :])
                        nc.sync.dma_start(rhs[:, D:2 * D],
                                          textb_d[ch * P:(ch + 1) * P, :])
                        rhs_tiles[ch] = rhs
                    else:
                        rhs = rhs_tiles[ch]

                    pt = pts.pop(ch)
                    if debug and mb == 0 and ch == 0:
                        dbg_pt_sb = const.tile([P, MBS], F32, name="dbg_pt_sb")
                        nc.vector.tensor_copy(dbg_pt_sb[:], pt[:])
                        nc.sync.dma_start(dbg_pt[:, :], dbg_pt_sb[:])
                    first, last = (ch == 0), (ch == NCH - 1)

                    def o_mm(sub):
                        nc.tensor.matmul(o_ps[sub][:],
                                         lhsT=pt[:, sub * P:(sub + 1) * P],
                                         rhs=rhs[:], start=first, stop=last)

                    # Interleave fresh-weight MMs (S, rs) between pt-weight O
                    # MMs so every LDWEIGHTS hides under a full 512-col stream.
                    if sp_n is not None:
                        s_mm(mb, nxt, 0, sp_n)
                    o_mm(0)
                    if sp_n is not None:
                        s_mm(mb, nxt, 1, sp_n)
                        pts[nxt] = s_act(mb, nxt, sp_n)
                    o_mm(1)
                    o_mm(2)
                    o_mm(3)
                    # rs MMs have a 1-partition output (col_grp masked); the
                    # array reconfig costs ~94ns each way, so batch them.
                    pending_rs.append((ch, pt))
                    if len(pending_rs) == RS_GROUP or last:
                        for c2, pt2 in pending_rs:
                            nc.tensor.matmul(rs_ps[:], lhsT=ones_sb[:],
                                             rhs=pt2[:],
                                             start=(c2 == 0), stop=(c2 == NCH - 1))
                        pending_rs.clear()

                # ---- epilogue: rowsum row -> per-partition recips ----
                rs_sb = rec_pool.tile([1, MBS], F32, tag="rssb", name=f"rssb{mb}")
                nc.vector.tensor_copy(rs_sb[:], rs_ps[:])
                # transpose [1,512] -> [128,4] via K=1 matmuls against a 1x1
                # ones tile (start=True clears the whole bank: only sub 0 sets it)
                tr_ps = s_pool.tile([P, NSUB], F32, tag="s", name=f"tr{mb}")
                for sub in range(NSUB):
                    nc.tensor.matmul(tr_ps[:, sub:sub + 1],
                                     lhsT=rs_sb[0:1, sub * P:(sub + 1) * P],
                                     rhs=one_f32[:],
                                     start=(sub == 0), stop=(sub == NSUB - 1),
                                     skip_group_check=True)
                recip = rec_pool.tile([P, NSUB], F32, tag="recip",
                                      name=f"recip{mb}")
                nc.vector.reciprocal(recip[:], tr_ps[:])
                if debug and mb == 0:
                    rec_dbg = rec_pool.tile([P, NSUB], F32, tag="recdbg", name="recdbg")
                    nc.vector.tensor_copy(rec_dbg[:], tr_ps[:])
                    nc.sync.dma_start(dbg_rs[:, :], rec_dbg[:])
                for sub in range(NSUB):
                    osb = eout_pool.tile([P, 2 * D], F32, tag="eout",
                                         name=f"eout{mb}_{sub}")
                    nc.vector.tensor_scalar(
                        osb[:], o_ps[sub][:], recip[:, sub:sub + 1], NORM,
                        op0=mybir.AluOpType.mult, op1=mybir.AluOpType.mult)
                    row0 = mb * MBS + sub * P
                    nc.sync.dma_start(out_d[row0:row0 + P, :], osb[:])

    nc.compile()
    return nc


def kernel(img, text, Wq, Wk):
    img = np.ascontiguousarray(img, dtype=np.float32)
    text = np.ascontiguousarray(text, dtype=np.float32)

    if "nc" not in _CACHE:
        _CACHE["nc"] = _build_nc()
    nc = _CACHE["nc"]

    textT16 = np.ascontiguousarray(text.T.astype(np.float16))
    img_bf = img.astype(ml_dtypes.bfloat16)
    text_bf = text.astype(ml_dtypes.bfloat16)
    wq16 = np.asarray(Wq, dtype=np.float16)
    wk16 = np.asarray(Wk, dtype=np.float16)

    in_maps = []
    for c in range(N_CORES):
        in_maps.append({
            "img_bf16": img_bf,
            "text_bf16": text_bf,
            "textT_f16": textT16,
            "imgT_f16": np.ascontiguousarray(
                img[c * SLAB:(c + 1) * SLAB].T.astype(np.float16)),
            "Wq_f16": wq16,
            "Wk_f16": wk16,
        })

    res = run_bass_kernel_spmd(nc, in_maps, core_ids=list(range(N_CORES)),
                               **_CACHE.get("run_kwargs", {}))
    _CACHE["last_results"] = res
    out = np.concatenate([res.results[c]["out"] for c in range(N_CORES)], axis=0)
    return np.ascontiguousarray(out[:, :D]), np.ascontiguousarray(out[:, D:])


if __name__ == "__main__":
    rng = np.random.default_rng(0)
    img = rng.standard_normal((N, D), dtype=np.float32)
    text = rng.standard_normal((N, D), dtype=np.float32)
    sc = 1.0 / np.sqrt(D)
    Wq = rng.uniform(-sc, sc, (D, D)).astype(np.float32)
    Wk = rng.uniform(-sc, sc, (D, D)).astype(np.float32)
    oi, ot = kernel(img, text, Wq, Wk)
    print("out_img", oi.shape, oi.dtype, "out_text", ot.shape, ot.dtype)
